# revision 22
# baseline (speedup 1.0000x reference)
"""Trainium2 Bass kernel for nn_Attention_77043123355775.

Sharded GQA causal attention with RoPE: 8 NeuronCores as 2-way data
parallel (batch) x 4-way tensor parallel (heads). Each core computes its
4 Q heads / 2 KV heads for one batch entry and a partial output
projection (x[b] @ W)^T; the host sums the 4 partials per batch.

All matmuls run in plain bf16 (fp32 PSUM accumulate), ~4e-3 max relative
error. Weights stay resident in SBUF. Q/K projections batch two token
chunks per stationary-weight load to amortize LDWEIGHTS; score blocks are
processed in pairs sharing one PSUM tile / one exp. The softmax
reciprocal runs on the scalar engine as exp(-ln(x)) (the DVE reciprocal
is ~8 cyc/elem on a single partition) and is broadcast by the gpsimd
engine, so normalization never touches the PE queue.
"""
import math
import os
import sys

for _p in ("/opt/trn_rl_repo",):
    if _p not in sys.path:
        sys.path.insert(0, _p)

import ml_dtypes
import numpy as np

import concourse.bass as bass
import concourse.mybir as mybir
import concourse.tile as tile

from concourse.tile import add_dep_helper

dt = mybir.dt
AF = mybir.ActivationFunctionType


def build_attention_nc(S=2048, D=2048, NQ=4, NKV=2, HD=128, TC=512):
    assert HD == 128
    C = D // 128          # contraction chunks over features
    TB = S // 128         # 128-token blocks
    NTC = S // TC         # token chunks
    DB = D // 128         # output feature blocks
    CO = NQ               # contraction chunks for wo
    REP = NQ // NKV
    scale = 1.0 / math.sqrt(HD)

    nc = bass.Bass()

    xt = nc.dram_tensor("xt", [D, S], dt.bfloat16, kind="ExternalInput")
    wqp = nc.dram_tensor("wqp", [D, NQ * HD], dt.bfloat16, kind="ExternalInput")
    wkp = nc.dram_tensor("wkp", [D, NKV * HD], dt.bfloat16, kind="ExternalInput")
    wvp = nc.dram_tensor("wvp", [D, NKV * HD], dt.bfloat16, kind="ExternalInput")
    wot = nc.dram_tensor("wot", [NQ * HD, D], dt.bfloat16, kind="ExternalInput")
    csT = nc.dram_tensor("csT", [HD, S], dt.bfloat16, kind="ExternalInput")
    masks = nc.dram_tensor("masks", [4 * 128, TC], dt.bfloat16, kind="ExternalInput")
    outT = nc.dram_tensor("outT", [D, S], dt.float32, kind="ExternalOutput")

    with tile.TileContext(nc) as tc:
        with (
            tc.tile_pool(name="const", bufs=1) as constp,
            tc.tile_pool(name="tabs", bufs=1) as tabp,
            tc.tile_pool(name="acts", bufs=1) as actp,
            tc.tile_pool(name="chunkacts", bufs=1) as cap,
            tc.tile_pool(name="wres", bufs=1) as wsp,
            tc.tile_pool(name="xstream", bufs=2) as xsp,
            tc.tile_pool(name="wo", bufs=1) as wop,
            tc.tile_pool(name="scratch", bufs=3) as scr,
            tc.tile_pool(name="psum", bufs=1, space="PSUM") as psp,
        ):
            ones_t = constp.tile([128, 1], dt.bfloat16, tag="ones")
            nc.vector.memset(ones_t[:], 1.0)
            ones_row = constp.tile([1, 128], dt.float32, tag="ones_row")
            nc.vector.memset(ones_row[:], 1.0)

            cs_t = tabp.tile([HD, S], dt.bfloat16, tag="cs")
            nc.gpsimd.dma_start(cs_t[:], csT[:])
            cos_t = cs_t[0:HD // 2, :]
            sin_t = cs_t[HD // 2:HD, :]
            mask_t = [tabp.tile([128, TC], dt.bfloat16, tag=f"mask{i}", name=f"mask{i}") for i in range(4)]

            # weights resident in SBUF for the whole kernel
            wqk_t = [wsp.tile([128, C * HD], dt.bfloat16, tag=f"wqk{h}", name=f"wqk{h}")
                     for h in range(NQ + NKV)]
            wv_t = wsp.tile([128, C * NKV * HD], dt.bfloat16, tag="wv")
            wo_t = wop.tile([128, CO * D], dt.bfloat16, tag="wo")

            # K/V persist for the full sequence (written chunk by chunk)
            kth = [actp.tile([128, S], dt.bfloat16, tag=f"kth{h}", name=f"kth{h}") for h in range(NKV)]
            vh_t = [actp.tile([128, NKV * HD], dt.bfloat16, tag=f"vh{b}", name=f"vh{b}") for b in range(TB)]
            # q / output tiles: two chunk-parities of q live at once (the
            # QKV phase projects a chunk pair before attention runs)
            qth = [[cap.tile([128, TC], dt.bfloat16, tag=f"qth{h}p{par}", name=f"qth{h}p{par}")
                    for par in range(2)] for h in range(NQ)]
            oth = [cap.tile([128, TC], dt.bfloat16, tag=f"oth{h}", name=f"oth{h}") for h in range(NQ)]

            wsrcs = [wqp] * NQ + [wkp] * NKV
            wcols = [h * HD for h in range(NQ)] + [h * HD for h in range(NKV)]

            def dma_wqk(h, split=False):
                src = wsrcs[h][:, wcols[h]:wcols[h] + HD]
                if not split:
                    nc.sync.dma_start(
                        wqk_t[h].rearrange("p (c n) -> p c n", c=C),
                        src.rearrange("(c p) n -> p c n", p=128),
                    )
                    return
                # first-needed weights in quarter pieces so the DMA-completion
                # semaphore for the first matmul's slice fires ~4x sooner
                # under the startup HBM burst
                for cq in range(4):
                    cs_ = slice(cq * (C // 4), (cq + 1) * (C // 4))
                    nc.sync.dma_start(
                        wqk_t[h][:, cs_.start * HD:cs_.stop * HD].rearrange(
                            "p (c n) -> p c n", c=C // 4),
                        src[cs_.start * 128:cs_.stop * 128, :].rearrange(
                            "(c p) n -> p c n", p=128),
                    )

            CQ = max(C // 4, 1)
            NG = C // CQ

            def rope_emit(h, ti, ps_half):
                # bf16 RoPE straight from the PSUM half into qth/kth
                ts = slice(ti * TC, (ti + 1) * TC)
                cs = cos_t[:, ts]
                sn = sin_t[:, ts]
                xr = ps_half[0:64, :]
                xi = ps_half[64:128, :]
                tA = scr.tile([128, TC], dt.bfloat16, tag="ropeA", bufs=2,
                              name=f"ropeA_{ti}_{h}")
                tB = scr.tile([128, TC], dt.bfloat16, tag="ropeB", bufs=2,
                              name=f"ropeB_{ti}_{h}")
                if h < NQ:
                    dst = qth[h][ti % 2][:]
                else:
                    dst = kth[h - NQ][:, ts]
                nc.vector.tensor_tensor(tA[0:64, :], xr, cs, mybir.AluOpType.mult)
                nc.vector.tensor_tensor(tB[0:64, :], xi, sn, mybir.AluOpType.mult)
                nc.vector.tensor_tensor(dst[0:64, :], tA[0:64, :], tB[0:64, :], mybir.AluOpType.subtract)
                nc.vector.tensor_tensor(tA[64:128, :], xr, sn, mybir.AluOpType.mult)
                nc.vector.tensor_tensor(tB[64:128, :], xi, cs, mybir.AluOpType.mult)
                nc.vector.tensor_tensor(dst[64:128, :], tA[64:128, :], tB[64:128, :], mybir.AluOpType.add)

            x_g = {}

            def emit_x_pair(P, ring, split_g0=False, gate=None):
                # 8 quarter-DMAs for chunk pair P, interleaved by chunk so
                # the pair matmuls unblock in c order. split_g0 loads the
                # first quarter c-chunk-wise (startup latency); gate defers
                # the first DMA behind a matmul so a prefetch burst stays
                # out of the startup HBM window.
                t0, t1 = 2 * P, 2 * P + 1
                x_g[t0] = []
                x_g[t1] = []
                for g in range(NG):
                    rs = slice(g * CQ * 128, (g + 1) * CQ * 128)
                    tiles = {}
                    for ti in (t0, t1):
                        tiles[ti] = xsp.tile([128, CQ * TC], dt.bfloat16, tag="xq", bufs=4 * NG,
                                             name=f"x_{ti}_{g}")
                        x_g[ti].append(tiles[ti])
                    if g == 0 and split_g0:
                        for ci in range(CQ):
                            rsc = slice(ci * 128, (ci + 1) * 128)
                            for ti in (t0, t1):
                                ring.dma_start(
                                    tiles[ti][:, ci * TC:(ci + 1) * TC],
                                    xt[rsc, ti * TC:(ti + 1) * TC],
                                )
                        continue
                    for ti in (t0, t1):
                        dma = ring.dma_start(
                            tiles[ti].rearrange("p (c n) -> p c n", c=CQ),
                            xt[rs, ti * TC:(ti + 1) * TC].rearrange("(c p) n -> p c n", p=128),
                        )
                        if gate is not None:
                            add_dep_helper(dma.ins, gate.ins,
                                           reason="defer prefetch past startup burst")
                            gate = None

            for P in range(NTC // 2):
                t0, t1 = 2 * P, 2 * P + 1
                # ---- input streams for chunk pair P ----
                # Rings: sync carries wqk, then pair-1 x (emitted mid pair
                # 0, off the startup window so the first-needed transfers
                # aren't sharing HBM with a prefetch burst); scalar carries
                # pair-0 x in parallel with the wqk loads; gpsimd carries
                # tables/wv/wo + output stores.
                if P == 0:
                    for h in range(NQ + NKV):
                        dma_wqk(h, split=(h == 0))
                    emit_x_pair(0, nc.scalar, split_g0=True)
                    nc.gpsimd.dma_start(
                        wv_t.rearrange("p (c n) -> p c n", c=C),
                        wvp.rearrange("(c p) n -> p c n", p=128),
                    )
                    for i in range(4):
                        nc.gpsimd.dma_start(mask_t[i][:], masks[i * 128:(i + 1) * 128, :])

                def xh_c(ti, c):
                    return x_g[ti][c // CQ][:, (c % CQ) * TC:(c % CQ + 1) * TC]

                # ---- Q/K projections for both chunks (one LDW per (h,c))
                qk_first_mm = {}
                for h in range(NQ + NKV):
                    ps = psp.tile([128, 2 * TC], dt.float32, tag="mm", bufs=2)
                    for c in range(C):
                        wht = wqk_t[h][:, c * HD:(c + 1) * HD]
                        mm = nc.tensor.matmul(ps[:, 0:TC], wht, xh_c(t0, c),
                                              start=(c == 0), stop=(c == C - 1))
                        if c == 0:
                            qk_first_mm[h] = mm
                        nc.tensor.matmul(ps[:, TC:2 * TC], wht, xh_c(t1, c),
                                         start=(c == 0), stop=(c == C - 1))
                    rope_emit(h, t0, ps[:, 0:TC])
                    rope_emit(h, t1, ps[:, TC:2 * TC])

                # ---- V projection ----
                for ti in (t0, t1):
                    for tb in range(TC // 128):
                        tbg = ti * (TC // 128) + tb
                        ps = psp.tile([128, NKV * HD], dt.float32, tag="mm", bufs=2,
                                      name=f"vps_{tbg}")
                        for c in range(C):
                            xh_s = xh_c(ti, c)[:, tb * 128:(tb + 1) * 128]
                            vht = wv_t[:, c * NKV * HD:(c + 1) * NKV * HD]
                            nc.tensor.matmul(ps[:], xh_s, vht,
                                             start=(c == 0), stop=(c == C - 1))
                        nc.vector.tensor_copy(vh_t[tbg][:], ps[:])

                # ---- attention + output projection per chunk ----
                for ti in (t0, t1):
                    if ti == 0:
                        nc.gpsimd.dma_start(
                            wo_t.rearrange("p (c n) -> p c n", c=CO),
                            wot.rearrange("(c p) n -> p c n", p=128),
                        )
                        emit_x_pair(1, nc.sync, gate=qk_first_mm[2])
                    qc = ti
                    npair = (qc + 1) * (TC // 256)
                    qpar = ti % 2

                    # Normalization is split: normA (ACT-only 1/sum via
                    # exp(-ln(x)), emitted at head end) and normB (the PE
                    # broadcast matmul + copy + scale, deferred into the
                    # NEXT head's stream so rec is ready when the PE
                    # reaches the broadcast - no PE stall).
                    norm_pend = {}
                    # all 4 heads' softmax sums share ONE psum bank, head h
                    # on partition row 32h (distinct col-groups, so the sum
                    # matmuls pack into the array and never contend on a
                    # rotating slot)
                    sums4 = psp.tile([128, TC], dt.float32, tag="sums", bufs=1,
                                     name=f"sums4_{ti}")

                    def emit_normA(h, ot_ps):
                        lg = scr.tile([1, TC], dt.float32, tag="lg", bufs=2,
                                      name=f"lg_{ti}_{h}")
                        nc.scalar.activation(lg[:], sums4[32 * h:32 * h + 1, :], AF.Ln,
                                             bias=0.0, scale=1.0)
                        rec = scr.tile([1, TC], dt.float32, tag="rec", bufs=2,
                                       name=f"rec_{ti}_{h}")
                        nc.scalar.activation(rec[:], lg[:], AF.Exp, bias=0.0, scale=-1.0)
                        norm_pend[h] = (ot_ps, rec)

                    def emit_normB(h):
                        ot_ps, rec = norm_pend.pop(h)
                        bc_ps = psp.tile([128, TC], dt.float32, tag="bcast", bufs=1,
                                         name=f"bc_{ti}_{h}")
                        nc.tensor.matmul(bc_ps[:], ones_row[:], rec[:], start=True, stop=True)
                        recb = scr.tile([128, TC], dt.float32, tag="recb", bufs=1,
                                        name=f"recb_{ti}_{h}")
                        nc.scalar.copy(recb[:], bc_ps[:])
                        nc.vector.tensor_tensor(oth[h][:], ot_ps[:], recb[:], mybir.AluOpType.mult)

                    LAGP = 2
                    head_ps = {}

                    def emit_scores(h, p):
                        kv = h // REP
                        kb0, kb1 = 2 * p, 2 * p + 1
                        d0 = kb0 * 128 - qc * TC
                        d1 = d0 + 128
                        q0 = max(d0, 0)
                        sc_ps = psp.tile([128, 2 * TC], dt.float32, tag="mm", bufs=2,
                                         name=f"sc_{ti}_{h}_{p}")
                        nc.tensor.matmul(sc_ps[:, q0:TC], kth[kv][:, kb0 * 128:(kb0 + 1) * 128],
                                         qth[h][qpar][:, q0:TC], start=True, stop=True)
                        nc.tensor.matmul(sc_ps[:, TC + q0:2 * TC], kth[kv][:, kb1 * 128:(kb1 + 1) * 128],
                                         qth[h][qpar][:, q0:TC], start=True, stop=True)
                        ph = scr.tile([128, 2 * TC], dt.bfloat16, tag="ph", bufs=LAGP + 2,
                                      name=f"ph_{ti}_{h}_{p}")
                        if q0 == 0:
                            nc.scalar.activation(ph[:, 0:2 * TC], sc_ps[:, 0:2 * TC],
                                                 AF.Exp, bias=0.0, scale=scale)
                        else:
                            nc.scalar.activation(ph[:, q0:TC], sc_ps[:, q0:TC],
                                                 AF.Exp, bias=0.0, scale=scale)
                            nc.scalar.activation(ph[:, TC + q0:2 * TC], sc_ps[:, TC + q0:2 * TC],
                                                 AF.Exp, bias=0.0, scale=scale)
                        if d0 >= 0:
                            nc.vector.tensor_tensor(ph[:, q0:TC], ph[:, q0:TC],
                                                    mask_t[d0 // 128][:, q0:TC], mybir.AluOpType.mult)
                        if d1 >= 0:
                            nc.vector.tensor_tensor(ph[:, TC + q0:2 * TC], ph[:, TC + q0:2 * TC],
                                                    mask_t[d1 // 128][:, q0:TC], mybir.AluOpType.mult)
                        return ph, q0

                    def emit_pv(h, p, ph, q0):
                        kv = h // REP
                        vcol = kv * HD
                        kb0, kb1 = 2 * p, 2 * p + 1
                        if p == 0:
                            head_ps[h] = psp.tile([128, TC], dt.float32, tag="otps", bufs=2,
                                                  name=f"ot_{ti}_{h}")
                        ot_ps = head_ps[h]
                        srow = sums4[32 * h:32 * h + 1, :]
                        last = p == npair - 1
                        nc.tensor.matmul(ot_ps[:, q0:TC], vh_t[kb0][:, vcol:vcol + HD],
                                         ph[:, q0:TC], start=(p == 0), stop=False)
                        nc.tensor.matmul(ot_ps[:, q0:TC], vh_t[kb1][:, vcol:vcol + HD],
                                         ph[:, TC + q0:2 * TC], start=False, stop=last)
                        nc.tensor.matmul(srow[:, q0:TC], ones_t[:],
                                         ph[:, q0:TC], start=(p == 0), stop=False,
                                         tile_position=(0, 32 * h))
                        nc.tensor.matmul(srow[:, q0:TC], ones_t[:],
                                         ph[:, TC + q0:2 * TC], start=False, stop=last,
                                         tile_position=(0, 32 * h))
                        if last:
                            emit_normA(h, ot_ps)

                    probs_q = []
                    for h in range(NQ):
                        for p in range(npair):
                            probs_q.append((h, p, *emit_scores(h, p)))
                            if len(probs_q) > LAGP:
                                emit_pv(*probs_q.pop(0))
                            # head h-1's broadcast matmul lands here, one
                            # score-pair into head h (after the lagged PV
                            # for h-1's last pair emitted normA): rec is
                            # ready by the time the PE reaches it
                            if p == min(1, npair - 1) and h >= 1 and h - 1 in norm_pend:
                                emit_normB(h - 1)
                    for args in probs_q:
                        emit_pv(*args)
                    for h in sorted(norm_pend):
                        if h != NQ - 1:
                            emit_normB(h)

                    # ---- output projection: feature-block groups rotate
                    # through FIVE psum slots (mm x2 wide + otps x2 +
                    # sums x1, all idle during this phase) with the c=3
                    # matmuls and o3 copies trailing three groups behind -
                    # the PE never waits on an o3 drain, and the last
                    # head's normB slots in after the first group.
                    ts_ = slice(ti * TC, (ti + 1) * TC)
                    slot_cycle = [("mm", 2, 2), ("mm", 2, 2), ("otps", 1, 2),
                                  ("otps", 1, 2), ("sums", 1, 1)]
                    groups = []
                    db = 0
                    gi = 0
                    while db < DB:
                        tag, w, bufs = slot_cycle[gi % len(slot_cycle)]
                        w = min(w, DB - db)
                        groups.append((tag, bufs, list(range(db, db + w))))
                        db += w
                        gi += 1
                    op_ps = {}

                    def oproj_c012(i):
                        tag, bufs, dbs = groups[i]
                        ps = psp.tile([128, len(dbs) * TC], dt.float32, tag=tag, bufs=bufs,
                                      name=f"ops_{ti}_{dbs[0]}")
                        op_ps[i] = ps
                        for half, db_ in enumerate(dbs):
                            for c in range(CO - 1):
                                wh_s = wo_t[:, c * D + db_ * 128:c * D + (db_ + 1) * 128]
                                nc.tensor.matmul(ps[:, half * TC:half * TC + TC], wh_s,
                                                 oth[c][:], start=(c == 0), stop=False)

                    def oproj_c3_store(i):
                        tag, bufs, dbs = groups[i]
                        ps = op_ps.pop(i)
                        for half, db_ in enumerate(dbs):
                            c = CO - 1
                            wh_s = wo_t[:, c * D + db_ * 128:c * D + (db_ + 1) * 128]
                            nc.tensor.matmul(ps[:, half * TC:half * TC + TC], wh_s,
                                             oth[c][:], start=False, stop=True)
                        o3 = scr.tile([128, len(dbs) * TC], dt.float32, tag="o3", bufs=4,
                                      name=f"o3_{ti}_{dbs[0]}")
                        eng_alt = [nc.scalar.copy, nc.vector.tensor_copy]
                        for half, db_ in enumerate(dbs):
                            eng_alt[(i + half) % 2](o3[:, half * TC:half * TC + TC],
                                                    ps[:, half * TC:half * TC + TC])
                        # last chunk: alternate store rings so the final
                        # drain splits across sync and gpsimd
                        if ti == NTC - 1:
                            deng = nc.sync if i % 2 == 0 else nc.gpsimd
                        else:
                            deng = nc.gpsimd
                        for half, db_ in enumerate(dbs):
                            deng.dma_start(outT[db_ * 128:(db_ + 1) * 128, ts_],
                                           o3[:, half * TC:half * TC + TC])

                    LAGO = 3
                    for i in range(len(groups)):
                        oproj_c012(i)
                        if i == 0:
                            emit_normB(NQ - 1)
                        if i >= LAGO:
                            oproj_c3_store(i - LAGO)
                    for i in range(len(groups) - LAGO, len(groups)):
                        oproj_c3_store(i)

    return nc


# ---------------------------------------------------------------------------
# walrus in this container refuses >1 sem wait per instruction ("Too many
# sync wait commands"). Hoist excess waits onto same-engine NoOps inserted
# immediately before the instruction - program order on the engine queue
# preserves the sync semantics.
def split_multiwait_insts(nc, max_waits=1):
    n_split = 0
    for bb in nc.main_func.blocks:
        insts = bb.instructions
        i = 0
        while i < len(insts):
            ins = insts[i]
            si = getattr(ins, "sync_info", None)
            if si is not None and si.on_wait and len(si.on_wait) > max_waits:
                waits = list(si.on_wait)
                head, tail = waits[:-max_waits], waits[-max_waits:]
                nops = []
                for j in range(0, len(head), max_waits):
                    nop = mybir.InstNoOp(name=f"{ins.name}-ws{j}", ins=[], outs=[])
                    nop.engine = ins.engine
                    nop.sync_info = mybir.SyncInfo(
                        on_wait=head[j:j + max_waits], on_update=[])
                    nops.append(nop)
                ins.sync_info = mybir.SyncInfo(
                    on_wait=tail, on_update=list(si.on_update or []))
                insts[i:i] = nops
                i += len(nops)
                n_split += 1
            i += 1
    return n_split


# ---------------------------------------------------------------------------
# Host-side shard preparation / gather
BF16 = ml_dtypes.bfloat16


def rope_tables(S, HD):
    inv = 1.0 / (10000.0 ** (np.arange(0, HD, 2, dtype=np.float32) / HD))
    t = np.arange(S, dtype=np.float32)
    f = np.outer(t, inv).astype(np.float32)  # [S, HD//2]
    return np.ascontiguousarray(np.cos(f).T), np.ascontiguousarray(np.sin(f).T)


def causal_masks(TC):
    # masks[dd][k, qrel] = 1 if k + dd*128 <= qrel else 0
    out = np.zeros((4 * 128, TC), BF16)
    k = np.arange(128)[:, None]
    q = np.arange(TC)[None, :]
    for dd in range(4):
        out[dd * 128:(dd + 1) * 128] = (k + dd * 128 <= q).astype(BF16)
    return out


def rope_perm(HD):
    # new row i (i < HD//2) = old 2i; new row HD//2+i = old 2i+1
    return np.concatenate([np.arange(0, HD, 2), np.arange(1, HD, 2)])


def make_in_maps(x, wq, wk, wv, wo, *, n_batch_shards, n_head_shards,
                 NQ_TOT, NKV_TOT, HD, TC):
    """Returns list of in_maps, one per core (batch-major: core = b*G + g)."""
    B, S, D = x.shape
    G = n_head_shards
    NQ = NQ_TOT // G
    NKV = NKV_TOT // G
    perm = rope_perm(HD)
    cosT, sinT = rope_tables(S, HD)
    csT = np.concatenate([cosT, sinT], axis=0).astype(BF16)  # [HD, S]
    masks = causal_masks(TC)

    # Per-batch xT (shared across head shards)
    xtm = {}
    for b in range(B):
        xtm[b] = np.ascontiguousarray(x[b].T).astype(BF16)  # [D, S]

    # Per-headgroup weight shards
    wshard = {}
    for g in range(G):
        qrows = slice(g * NQ * HD, (g + 1) * NQ * HD)
        kvrows = slice(g * NKV * HD, (g + 1) * NKV * HD)
        wq_g = wq[qrows, :].copy()      # [NQ*HD, D]
        wk_g = wk[kvrows, :].copy()
        wv_g = wv[kvrows, :].copy()
        # RoPE permutation of output rows, per head
        for hh in range(NQ):
            blk = wq_g[hh * HD:(hh + 1) * HD]
            wq_g[hh * HD:(hh + 1) * HD] = blk[perm]
        for hh in range(NKV):
            blk = wk_g[hh * HD:(hh + 1) * HD]
            wk_g[hh * HD:(hh + 1) * HD] = blk[perm]
        wqT = np.ascontiguousarray(wq_g.T).astype(BF16)   # [D, NQ*HD]
        wkT = np.ascontiguousarray(wk_g.T).astype(BF16)
        wvT = np.ascontiguousarray(wv_g.T).astype(BF16)
        woT = np.ascontiguousarray(wo[:, qrows].T).astype(BF16)  # [NQ*HD, D]
        wshard[g] = (wqT, wkT, wvT, woT)

    in_maps = []
    for b in range(n_batch_shards):
        for g in range(G):
            wqT, wkT, wvT, woT = wshard[g]
            in_maps.append({
                "xt": xtm[b],
                "wqp": wqT, "wkp": wkT, "wvp": wvT, "wot": woT,
                "csT": csT,
                "masks": masks,
            })
    return in_maps


def combine_outputs(outTs, B, G):
    """outTs: list of [D, S] partials, core order b*G+g. Returns [B, S, D]."""
    outs = []
    for b in range(B):
        acc = outTs[b * G].astype(np.float32).copy()
        for g in range(1, G):
            acc += outTs[b * G + g]
        outs.append(acc.T)  # [S, D]
    return np.stack(outs)


_NC_CACHE = {}


def _get_nc(S, D, NQ, NKV, HD, TC):
    key = (S, D, NQ, NKV, HD, TC)
    if key not in _NC_CACHE:
        nc = build_attention_nc(S=S, D=D, NQ=NQ, NKV=NKV, HD=HD, TC=TC)
        split_multiwait_insts(nc)
        _NC_CACHE[key] = nc
    return _NC_CACHE[key]


def kernel(**inputs):
    x = np.asarray(inputs["x"], dtype=np.float32)
    wq = np.asarray(inputs["wq"], dtype=np.float32)
    wk = np.asarray(inputs["wk"], dtype=np.float32)
    wv = np.asarray(inputs["wv"], dtype=np.float32)
    wo = np.asarray(inputs["wo"], dtype=np.float32)

    B, S, D = x.shape          # (2, 2048, 2048)
    NQ_TOT = wq.shape[0] // 128
    NKV_TOT = wk.shape[0] // 128
    HD = 128
    TC = 512
    G = 4                      # head shards
    NQ, NKV = NQ_TOT // G, NKV_TOT // G

    nc = _get_nc(S, D, NQ, NKV, HD, TC)
    in_maps = make_in_maps(
        x, wq, wk, wv, wo,
        n_batch_shards=B, n_head_shards=G,
        NQ_TOT=NQ_TOT, NKV_TOT=NKV_TOT, HD=HD, TC=TC,
    )

    from concourse.bass_utils import run_bass_kernel_spmd

    trace = os.environ.get("BASS_ATTN_TRACE") == "1"
    res = run_bass_kernel_spmd(nc, in_maps, list(range(len(in_maps))), trace=trace)
    kernel.last_results = res
    outTs = [r["outT"] for r in res.results]
    return combine_outputs(outTs, B, G).astype(np.float32)
